# revision 26
# baseline (speedup 1.0000x reference)
"""BandSplitLinear Trainium2 kernel (v3: fp16 PE datapath, PE transposes).

Strategy (per core, batch-parallel over 8 cores):
  - No nonlinearity between the two per-band linears -> fold w_pre @ w_post
    into one 128x128 matrix per band on the host (6x fewer FLOPs). Biases are
    additive constants per (c, f) -> applied host-side.
  - Bands are disjoint contiguous frequency ranges. Carve the frequency axis
    into 33 aligned segments of 32 bins; per segment use the 128-partition
    feature layout g = c*32 + u. Every band spans <= 2 segments, so the whole
    computation becomes y.T[seg_out] = sum_{seg_in} Wg[seg_in, seg_out].T @
    x.T[seg_in] over 97 host-built zero-padded 128x128 blocks. Gather/scatter
    vanish into the weight sparsity pattern.
  - fp16 datapath on chip (fp32 PSUM accumulation): SWDGE cast-DMA loads,
    DVE pack into segment-major layout, PE transposes (1 cyc/row at fp16),
    fp16 matmuls with resident weights, PE transposes back, strided copies
    into output staging, SWDGE cast-DMA stores.
"""

import numpy as np

import concourse.bass as bass
import concourse.tile as tile
from concourse import bacc, mybir
from concourse.bass_utils import run_bass_kernel_spmd
from concourse.masks import make_identity



# ---- problem constants (hardcoded per spec) ----
B, C, T, F = 8, 4, 1000, 1025
N_CORES = 8
SEG = 32
FOFF = 22  # grid phase: f + FOFF = 32*j + u; boundaries at f = 10 (mod 32)
NSEG = (F - 1 + FOFF) // SEG + 1  # 33
CPL = NSEG * SEG  # 1056, c-plane width in staging buffers
GW = NSEG * 128  # packed width: 4224
T_BLOCKS = [(0, 128), (128, 384), (512, 488)]
P = 128

_F32 = mybir.dt.float32
_F16 = mybir.dt.float16


def _build_bands():
    f, interval = 0, 4
    groups = []
    while f < F:
        end = min(f + interval, F)
        groups.append((f, end))
        f = end
        if interval < 32:
            interval += 1
    return groups  # list of (start, end), disjoint, covering [0, F)


def _block_structure():
    """Nonzero (j_out, j_in) block pairs, grouped by j_out (ascending j_in)."""
    bands = _build_bands()
    pairs = set()
    for start, end in bands:
        segs = set(range((start + FOFF) // SEG, (end - 1 + FOFF) // SEG + 1))
        for ji in segs:
            for jo in segs:
                pairs.add((jo, ji))
    jin_lists = [sorted(ji for (jo, ji) in pairs if jo == j) for j in range(NSEG)]
    return bands, jin_lists


def _build_weight_blocks(w_pre, w_post):
    """Host: fold per-band linears and scatter into segment-pair blocks."""
    bands, jin_lists = _block_structure()
    wc = np.einsum(
        "kio,kod->kid", w_pre.astype(np.float64), w_post.astype(np.float64)
    )  # [45, 128, 128], both feature dims indexed by w*4 + c
    blocks = {}
    for k, (start, end) in enumerate(bands):
        fs = np.arange(start, end)
        js = (fs + FOFF) // SEG
        us = (fs + FOFF) % SEG
        for ji in np.unique(js):
            for jo in np.unique(js):
                key = (int(jo), int(ji))
                if key not in blocks:
                    blocks[key] = np.zeros((P, P), dtype=np.float64)
                blk = blocks[key]
                mi = js == ji
                mo = js == jo
                wi = fs[mi] - start
                wo = fs[mo] - start
                for ci in range(C):
                    for co in range(C):
                        blk[np.ix_(ci * SEG + us[mi], co * SEG + us[mo])] = wc[k][
                            np.ix_(wi * C + ci, wo * C + co)
                        ]
    order = [(jo, ji) for jo in range(NSEG) for ji in jin_lists[jo]]
    wall = np.stack([blocks[key] for key in order]).astype(np.float16)
    offs = np.cumsum([0] + [len(jl) for jl in jin_lists])
    return wall, jin_lists, offs


def _bias_field(bands, b_pre, w_post, b_post):
    """bias[c, f]: the constant added to out[., c, ., f]."""
    bc = (
        np.einsum("ko,kod->kd", b_pre.astype(np.float64), w_post.astype(np.float64))
        + b_post.astype(np.float64)
    )
    field = np.zeros((C, F), dtype=np.float64)
    for k, (start, end) in enumerate(bands):
        for c in range(C):
            field[c, start:end] = bc[k, (np.arange(end - start)) * C + c]
    return field.astype(np.float32)


def _t_chunks(t0, tlen):
    out = []
    off = 0
    while off < tlen:
        n = min(P, tlen - off)
        out.append((t0 + off, off, n))
        off += n
    return out


def _build_nc(jin_lists, offs, nblk):
    nc = bacc.Bacc("TRN2", target_bir_lowering=False, debug=False)
    xs = nc.dram_tensor("xs", [C, T, F], _F32, kind="ExternalInput")
    wall = nc.dram_tensor("wall", [nblk, P, P], _F16, kind="ExternalInput")
    ys = nc.dram_tensor("ys", [C, T, F], _F32, kind="ExternalOutput")

    with tile.TileContext(nc) as tc:
        import contextlib

        ctx = contextlib.ExitStack()
        with ctx:
            const_pool = ctx.enter_context(tc.tile_pool(name="const", bufs=1))
            stg_pool = ctx.enter_context(tc.tile_pool(name="stg", bufs=2))
            packed_pool = ctx.enter_context(tc.tile_pool(name="packed", bufs=7))
            ystga_pool = ctx.enter_context(tc.tile_pool(name="ystga", bufs=5))
            ystgb_pool = ctx.enter_context(tc.tile_pool(name="ystgb", bufs=5))
            at_pool = ctx.enter_context(tc.tile_pool(name="atseg", bufs=8))
            yt_pool = ctx.enter_context(tc.tile_pool(name="ytseg", bufs=6))
            ps_at_pool = ctx.enter_context(
                tc.tile_pool(name="psat", bufs=2, space="PSUM")
            )
            ps_y_pool = ctx.enter_context(
                tc.tile_pool(name="psy", bufs=2, space="PSUM")
            )
            ps_o_pool = ctx.enter_context(
                tc.tile_pool(name="pso", bufs=2, space="PSUM")
            )

            ident = const_pool.tile([P, P], _F16)
            make_identity(nc, ident[:])


            # resident fp16 weights: [128, nblk*128]
            wall_sb = const_pool.tile([P, nblk * P], _F16)
            nc.scalar.dma_start(
                wall_sb[:].rearrange("p (n o) -> p n o", o=P),
                wall.ap().rearrange("n p o -> p n o"),
            )

            def load_and_pack(t0, tlen):
                packed = []
                for tglob, toff, ntc in _t_chunks(t0, tlen):
                    stg = stg_pool.tile([P, C * CPL], _F32, name="stg")
                    for c in range(C):
                        nc.sync.dma_start(
                            stg[0:ntc, c * CPL : c * CPL + F],
                            xs.ap()[c, tglob : tglob + ntc, :],
                        )
                        nc.gpsimd.memset(stg[0:ntc, c * CPL + F : (c + 1) * CPL], 0.0)
                    pk = packed_pool.tile([P, GW], _F16, name="pk")
                    # seg 0 covers f in [-FOFF, SEG-FOFF): zero the pad rows
                    nc.gpsimd.memset(pk[0:ntc, 0:P], 0.0)
                    for c in range(C):
                        # seg 0: f 0..SEG-FOFF-1 at u FOFF..SEG-1
                        nc.vector.tensor_copy(
                            pk[0:ntc, c * SEG + FOFF : (c + 1) * SEG],
                            stg[0:ntc, c * CPL : c * CPL + SEG - FOFF],
                        )
                        # segs 1..NSEG-1: f contiguous from SEG-FOFF
                        src = stg[
                            0:ntc,
                            c * CPL + SEG - FOFF : c * CPL + SEG - FOFF
                            + (NSEG - 1) * SEG,
                        ].rearrange("p (j u) -> p j u", u=SEG)
                        dst = pk[0:ntc, P:].rearrange(
                            "p (j cc u) -> p j cc u", cc=C, u=SEG
                        )[:, :, c, :]
                        nc.vector.tensor_copy(dst, src)
                    packed.append((pk, toff, ntc))
                return packed

            packed_next = load_and_pack(*T_BLOCKS[0])
            for bi, (t0, tlen) in enumerate(T_BLOCKS):
                chunks = _t_chunks(t0, tlen)
                packed = packed_next
                if bi + 1 < len(T_BLOCKS):
                    packed_next = load_and_pack(*T_BLOCKS[bi + 1])

                ystga = {}
                ystgb = {}
                for _tglob, toff, ntc in chunks:
                    ystga[toff] = ystga_pool.tile([P, C * 490], _F32, name="ystga")
                    ystgb[toff] = ystgb_pool.tile(
                        [P, C * (CPL - 490)], _F16, name="ystgb"
                    )

                # ---- per-segment pipeline ----
                at_segs = {}

                def ensure_seg(j, packed=packed, at_segs=at_segs, tlen=tlen):
                    if j in at_segs:
                        return
                    ps = ps_at_pool.tile([P, 512], _F16, name="psat")
                    for pk, toff, ntc in packed:
                        nc.tensor.transpose(
                            ps[:, toff : toff + ntc],
                            pk[0:ntc, j * P : (j + 1) * P],
                            ident[0:ntc, 0:ntc],
                        )
                    seg = at_pool.tile([P, 512], _F16, name="atseg")
                    if j % 2 == 0:
                        nc.scalar.copy(seg[:, 0:tlen], ps[:, 0:tlen])
                    else:
                        nc.vector.tensor_copy(seg[:, 0:tlen], ps[:, 0:tlen])
                    at_segs[j] = seg

                ytiles = {}
                for j_out in range(NSEG):
                    jins = jin_lists[j_out]
                    nw = len(jins)
                    for j in jins:
                        ensure_seg(j)
                    psy = ps_y_pool.tile([P, 512], _F32, name="psy")
                    w0 = offs[j_out]
                    for i, j in enumerate(jins):
                        nc.tensor.matmul(
                            psy[:, 0:tlen],
                            lhsT=wall_sb[:, (w0 + i) * P : (w0 + i + 1) * P],
                            rhs=at_segs[j][:, 0:tlen],
                            start=(i == 0),
                            stop=(i == nw - 1),
                        )
                    yt = yt_pool.tile([P, 512], _F16, name="ytseg")
                    nc.scalar.copy(yt[:, 0:tlen], psy[:, 0:tlen])
                    ytiles[j_out] = yt

                    # ---- flush group of 4 output segments ----
                    last_in_group = (j_out % 4 == 3) or (j_out == NSEG - 1)
                    if not last_in_group:
                        continue
                    g0 = (j_out // 4) * 4
                    gn = j_out - g0 + 1
                    for _tglob, toff, ntc in chunks:
                        pso = ps_o_pool.tile([P, 512], _F16, name="pso")
                        for jj in range(gn):
                            nc.tensor.transpose(
                                pso[0:ntc, jj * P : (jj + 1) * P],
                                ytiles[g0 + jj][:, toff : toff + ntc],
                                ident[:],
                            )
                        # groups 0-3 cover f<490 -> fp32 staging (HWDGE store);
                        # groups 4-8 cover f>=490 -> fp16 staging (SWDGE cast store)
                        if g0 <= 12:
                            ys_t = ystga[toff]
                            ysr = ys_t[0:ntc].rearrange("p (cc x) -> p cc x", cc=C)
                            fbase = 0
                        else:
                            ys_t = ystgb[toff]
                            ysr = ys_t[0:ntc].rearrange("p (cc x) -> p cc x", cc=C)
                            fbase = 490
                        if g0 == 0:
                            # seg 0: valid u FOFF.. -> f 0..SEG-FOFF-1
                            nc.vector.tensor_copy(
                                ysr[:, :, 0 : SEG - FOFF],
                                pso[0:ntc, 0:P].rearrange(
                                    "p (cc u) -> p cc u", cc=C
                                )[:, :, FOFF:SEG],
                            )
                            src = pso[0:ntc, P : gn * P].rearrange(
                                "p (jj cc u) -> p jj cc u", cc=C, u=SEG
                            )
                            dst = ysr[
                                :, :, SEG - FOFF : SEG - FOFF + (gn - 1) * SEG
                            ].rearrange("p cc (j u) -> p j cc u", u=SEG)
                            nc.vector.tensor_copy(dst, src)
                        elif g0 + gn - 1 == NSEG - 1:
                            uvalid = F - (SEG * (NSEG - 1) - FOFF)
                            f0 = SEG * (NSEG - 1) - FOFF - fbase
                            nc.vector.tensor_copy(
                                ysr[:, :, f0 : f0 + uvalid],
                                pso[0:ntc, 0:P].rearrange(
                                    "p (cc u) -> p cc u", cc=C
                                )[:, :, 0:uvalid],
                            )
                        else:
                            src = pso[0:ntc, 0 : gn * P].rearrange(
                                "p (jj cc u) -> p jj cc u", cc=C, u=SEG
                            )
                            f0 = SEG * g0 - FOFF - fbase
                            dst = ysr[:, :, f0 : f0 + gn * SEG].rearrange(
                                "p cc (j u) -> p j cc u", u=SEG
                            )
                            nc.vector.tensor_copy(dst, src)
                        tglob_c = t0 + toff
                        if g0 == 12 and gn == 4:
                            # f<490 complete: big fp32 store on the sync ring
                            for c in range(C):
                                nc.sync.dma_start(
                                    ys.ap()[c, tglob_c : tglob_c + ntc, 0:490],
                                    ystga[toff][0:ntc, c * 490 : (c + 1) * 490],
                                )
                        if g0 == 24 and gn == 4:
                            # 490 <= f < 874 complete: SWDGE cast store
                            for c in range(C):
                                nc.gpsimd.dma_start(
                                    ys.ap()[c, tglob_c : tglob_c + ntc, 490:874],
                                    ystgb[toff][0:ntc, c * 566 : c * 566 + 384],
                                )
                # ---- store the final f-sliver (cast fp16->fp32) ----
                for tglob, toff, ntc in chunks:
                    for c in range(C):
                        nc.gpsimd.dma_start(
                            ys.ap()[c, tglob : tglob + ntc, 874:F],
                            ystgb[toff][0:ntc, c * 566 + 384 : c * 566 + 535],
                        )
    nc.compile()
    return nc


_CACHE = {}


def kernel(x, w_pre, b_pre, w_post, b_post):
    x = np.asarray(x, dtype=np.float32)
    w_pre = np.asarray(w_pre, dtype=np.float32)
    b_pre = np.asarray(b_pre, dtype=np.float32)
    w_post = np.asarray(w_post, dtype=np.float32)
    b_post = np.asarray(b_post, dtype=np.float32)

    bands, _ = _block_structure()
    wall, jin_lists, offs = _build_weight_blocks(w_pre, w_post)
    nblk = wall.shape[0]

    if "nc" not in _CACHE:
        _CACHE["nc"] = _build_nc(jin_lists, offs, nblk)
    nc = _CACHE["nc"]

    in_maps = [{"xs": x[b], "wall": wall} for b in range(N_CORES)]
    res = run_bass_kernel_spmd(nc, in_maps, core_ids=list(range(N_CORES)))
    out = np.stack([res.results[b]["ys"] for b in range(N_CORES)])

    if np.any(b_pre) or np.any(b_post):
        field = _bias_field(bands, b_pre, w_post, b_post)
        out = out + field[None, :, None, :]
    return out
